# revision 23
# baseline (speedup 1.0000x reference)
"""Trainium2 Bass kernel for nn_ACDMNET (dense_mlp, 8 NeuronCores).

Math (per reference):
    A1[b,e] = sum_d stu_v[b,d] * |W1|[e,d]        (first half of W1)
    C1[k,e] = sum_d kn[k,d]    * |W1|[e,d+128] + b1[e]
    A2,C2 likewise from exer_v / W2, b2
    g[b,e]  = sigmoid(stu_q[b,e]*exer_k[b,e])     (disc)
    opre[b,k] = sum_e (sig(A1+C1) - sig(A2+C2)) * g[b,e] * |W3|[e]
    o = sig(opre + b3);  out[b] = sum_k o*kq / sum_k kq

Sharding: the knowledge axis K=128 is split 16-per-core so the dominant
sigmoid work runs as long-free-dim activation instructions; each core
gathers all 4096 embedding rows (replicated tables) and emits partial
(sum, count) rows; the host sums partials across cores and divides.

The embedding gather dominates the serial head (SWDGE descriptor emission
~8ns/row on GpSimd), so student_v/student_q (and exercise_v/exercise_k) are
fused host-side into (20000, 256) bf16 tables: one dma_gather descriptor
fetches both, and transpose-mode lands them as (128, 2, n) = (vT, qT).
Gathers and the main loop are split into two batch halves, pipelining the
second half's gather under the first half's compute.
"""

import os
from contextlib import ExitStack

import numpy as np
import ml_dtypes

B = 4096          # batch
E = 128           # embedding dim
K = 128           # knowledge concepts
NCORES = 8
KL = K // NCORES  # 16 concepts per core
TBL = 20000       # table rows
BH = B // 2       # half-batch
NBH = BH // 512   # 512-wide PSUM chunks per half

_CACHE = {}
LAST_RESULTS = None  # BassKernelResults of the most recent run (for profiling)


def _build():
    import concourse.bass as bass
    import concourse.tile as tile
    from concourse import bacc, mybir

    bf16 = mybir.dt.bfloat16
    f32 = mybir.dt.float32
    i16 = mybir.dt.int16
    AF = mybir.ActivationFunctionType
    OP = mybir.AluOpType

    nc = bacc.Bacc("TRN2", target_bir_lowering=False, debug=False,
                   num_devices=NCORES)

    def din(name, shape, dt):
        return nc.dram_tensor(name, shape, dt, kind="ExternalInput").ap()

    t_stu = din("stu", [TBL, 2 * E], bf16)   # [student_v | student_q]
    t_exer = din("exer", [TBL, 2 * E], bf16)  # [exercise_v | exercise_k]
    d_w1aT = din("w1aT", [E, E], bf16)
    d_w1bT = din("w1bT", [E, E], bf16)
    d_w2aT = din("w2aT", [E, E], bf16)
    d_w2bT = din("w2bT", [E, E], bf16)
    d_w3T = din("w3T", [E, 1], bf16)
    d_knT = din("knT", [E, KL], bf16)
    d_b1 = din("b1", [E, 1], f32)
    d_b2 = din("b2", [E, 1], f32)
    d_b3 = din("b3t", [KL, 1], f32)
    d_kqT = din("kqT", [KL, B], bf16)
    d_idxS = din("idxS", [128, B // 16], i16)
    d_idxE = din("idxE", [128, B // 16], i16)
    d_out = nc.dram_tensor("out", [2, B], f32, kind="ExternalOutput").ap()

    with tile.TileContext(nc) as tc, ExitStack() as ctx:
        sing = ctx.enter_context(tc.tile_pool(name="sing", bufs=1))
        work = ctx.enter_context(tc.tile_pool(name="work", bufs=3))

        # ---- constant loads -------------------------------------------------
        def load(name, ap, shape, dt):
            t = sing.tile(shape, dt, tag=name)
            nc.sync.dma_start(t[:], ap)
            return t

        idxS = load("idxS", d_idxS, [128, B // 16], i16)
        idxE = load("idxE", d_idxE, [128, B // 16], i16)
        w1aT = load("w1aT", d_w1aT, [E, E], bf16)
        w1bT = load("w1bT", d_w1bT, [E, E], bf16)
        w2aT = load("w2aT", d_w2aT, [E, E], bf16)
        w2bT = load("w2bT", d_w2bT, [E, E], bf16)
        w3T = load("w3T", d_w3T, [E, 1], bf16)
        knT = load("knT", d_knT, [E, KL], bf16)
        b1 = load("b1", d_b1, [E, 1], f32)
        b2 = load("b2", d_b2, [E, 1], f32)
        b3t = load("b3t", d_b3, [KL, 1], f32)
        kqTt = sing.tile([KL, B], bf16, tag="kqTt")
        nc.sync.dma_start(kqTt[:], d_kqT)

        # ---- transposed gathers, half 0 first (main loop pipelines on it) ---
        # transpose-mode dma_gather of a 256-wide row lands (128, 2, n):
        # [:, 0, :] = v-part transposed, [:, 1, :] = q/k-part transposed.
        stu_g = []
        exer_g = []
        for h in range(2):
            sg = sing.tile([E, 2, BH], bf16, tag=f"stu_g{h}")
            eg = sing.tile([E, 2, BH], bf16, tag=f"exer_g{h}")
            stu_g.append(sg)
            exer_g.append(eg)
        for h in range(2):
            isl = slice(h * (BH // 16), (h + 1) * (BH // 16))
            nc.gpsimd.dma_gather(
                out_ap=stu_g[h][:], in_ap=t_stu, idxs_ap=idxS[:, isl],
                num_idxs=BH, num_idxs_reg=BH, elem_size=2 * E, transpose=True,
                single_packet=False)
            nc.gpsimd.dma_gather(
                out_ap=exer_g[h][:], in_ap=t_exer, idxs_ap=idxE[:, isl],
                num_idxs=BH, num_idxs_reg=BH, elem_size=2 * E, transpose=True,
                single_packet=False)

        # ---- |W| (DVE, abs_max against 0) -----------------------------------
        w1aTa = sing.tile([E, E], bf16, tag="w1aTa")
        w1bTa = sing.tile([E, E], bf16, tag="w1bTa")
        w2aTa = sing.tile([E, E], bf16, tag="w2aTa")
        w2bTa = sing.tile([E, E], bf16, tag="w2bTa")
        w3Ta = sing.tile([E, 1], bf16, tag="w3Ta")
        for dst, src in ((w1aTa, w1aT), (w1bTa, w1bT), (w2aTa, w2aT),
                         (w2bTa, w2bT), (w3Ta, w3T)):
            nc.scalar.activation(dst[:], src[:], AF.Abs)

        # one-hot-scaled lhsT bank: w3oh[:, k, j] = |w3|[e] * (j == k)
        w3oh = sing.tile([E, KL, KL], bf16, tag="w3oh")
        nc.vector.memset(w3oh[:], 0.0)
        for k in range(KL):
            nc.vector.tensor_copy(w3oh[:, k, k:k + 1], w3Ta[:])
        ones16 = sing.tile([KL, 1], bf16, tag="ones16")
        nc.vector.memset(ones16[:], 1.0)

        out_sb = sing.tile([33, B], f32, tag="out_sb")

        A1t = sing.tile([E, B], bf16, tag="A1t")
        A2t = sing.tile([E, B], bf16, tag="A2t")
        C1t = sing.tile([E, KL], f32, tag="C1t")
        C2t = sing.tile([E, KL], f32, tag="C2t")
        gT = sing.tile([E, B], bf16, tag="gT")

        psA_cm = tc.tile_pool(name="psA", bufs=2, space="PSUM")
        psA = psA_cm.__enter__()

        # C1t/C2t (no gather dependency — runs during gather emission)
        cps = psA.tile([E, KL], f32, tag="mm")
        nc.tensor.matmul(out=cps[:], lhsT=w1bTa[:], rhs=knT[:],
                         start=True, stop=True)
        nc.vector.tensor_scalar_add(C1t[:], cps[:], b1[:])
        cps2 = psA.tile([E, KL], f32, tag="mm")
        nc.tensor.matmul(out=cps2[:], lhsT=w2bTa[:], rhs=knT[:],
                         start=True, stop=True)
        nc.vector.tensor_scalar_add(C2t[:], cps2[:], b2[:])

        # count[b] = sum_k kq (local part) — also gather-independent
        for c in range(B // 512):
            ch = slice(c * 512, (c + 1) * 512)
            cnt = psA.tile([1, 512], f32, tag="mm")
            nc.tensor.matmul(out=cnt[:], lhsT=ones16[:], rhs=kqTt[:, ch],
                             start=True, stop=True)
            nc.scalar.copy(out_sb[32:33, ch], cnt[:])

        # ---- per-half: A1t/A2t projections + gT -----------------------------
        def setup_half(h):
            hs = slice(h * BH, (h + 1) * BH)
            for dst, lhs, g3 in ((A1t, w1aTa, stu_g[h]), (A2t, w2aTa, exer_g[h])):
                for c in range(NBH):
                    ps = psA.tile([E, 512], f32, tag="mm")
                    nc.tensor.matmul(out=ps[:], lhsT=lhs[:],
                                     rhs=g3[:, 0, c * 512:(c + 1) * 512],
                                     start=True, stop=True)
                    nc.vector.tensor_copy(
                        dst[:, h * BH + c * 512:h * BH + (c + 1) * 512], ps[:])
            mT = work.tile([E, BH], bf16, tag="mT")
            nc.vector.tensor_tensor(out=mT[:], in0=stu_g[h][:, 1, :],
                                    in1=exer_g[h][:, 1, :], op=OP.mult)
            nc.scalar.activation(gT[:, hs], mT[:], AF.Sigmoid)

        setup_half(0)
        setup_half(1)
        psA_cm.__exit__(None, None, None)

        # ---- main loop: half-outer, k-inner ----------------------------------
        psB_cm = tc.tile_pool(name="psB", bufs=2, space="PSUM")
        psB = psB_cm.__enter__()

        def main_half(h):
            hs = slice(h * BH, (h + 1) * BH)
            opre = psB.tile([KL, BH], f32, tag="big")
            for k in range(KL):
                pk = work.tile([E, BH], bf16, tag="pk")
                dk = work.tile([E, BH], bf16, tag="dk")
                nc.scalar.activation(pk[:], A1t[:, hs], AF.Sigmoid,
                                     bias=C1t[:, k:k + 1])
                nc.scalar.activation(dk[:], A2t[:, hs], AF.Sigmoid,
                                     bias=C2t[:, k:k + 1])
                tk = work.tile([E, BH], bf16, tag="tk")
                nc.vector.tensor_tensor(out=tk[:], in0=pk[:], in1=dk[:],
                                        op=OP.subtract)
                wk = work.tile([E, BH], bf16, tag="wk")
                nc.vector.tensor_tensor(out=wk[:], in0=tk[:], in1=gT[:, hs],
                                        op=OP.mult)
                for c in range(NBH):
                    nc.tensor.matmul(
                        out=opre[:, c * 512:(c + 1) * 512],
                        lhsT=w3oh[:, k, :],
                        rhs=wk[:, c * 512:(c + 1) * 512],
                        start=(k == 0), stop=(k == KL - 1),
                        skip_group_check=True)
            return opre

        def tail_half(h, opre):
            hs = slice(h * BH, (h + 1) * BH)
            o = work.tile([KL, BH], bf16, tag="o")
            nc.scalar.activation(o[:], opre[:], AF.Sigmoid, bias=b3t[:])
            mo = work.tile([KL, BH], bf16, tag="mo")
            nc.vector.tensor_tensor(out=mo[:], in0=o[:], in1=kqTt[:, hs],
                                    op=OP.mult)
            osum = psB.tile([1, BH], f32, tag="big")
            for c in range(NBH):
                ch = slice(c * 512, (c + 1) * 512)
                nc.tensor.matmul(out=osum[:, ch], lhsT=ones16[:],
                                 rhs=mo[:, ch], start=True, stop=True)
            nc.vector.tensor_copy(out_sb[0:1, hs], osum[:])

        op0 = main_half(0)
        tail_half(0, op0)
        op1 = main_half(1)
        tail_half(1, op1)
        psB_cm.__exit__(None, None, None)

        nc.sync.dma_start(d_out[0:1, :], out_sb[0:1, :])
        nc.sync.dma_start(d_out[1:2, :], out_sb[32:33, :])

    nc.compile()
    return nc


def _wrap_idx(ids):
    # dma_gather index layout: idx i lives at [i % 16, i // 16], replicated
    # across the 8 16-partition groups.
    w = ids.astype(np.int16).reshape(B // 16, 16).T
    return np.ascontiguousarray(np.tile(w, (8, 1)))


def kernel(**inputs):
    from concourse.bass_utils import run_bass_kernel_spmd
    global LAST_RESULTS

    if "nc" not in _CACHE:
        _CACHE["nc"] = _build()
    nc = _CACHE["nc"]

    bf = ml_dtypes.bfloat16
    f32 = np.float32
    stu_id = np.asarray(inputs["stu_id"])
    exer_id = np.asarray(inputs["exer_id"])
    kq = np.asarray(inputs["kq"], dtype=f32)
    W1 = np.asarray(inputs["W1"], dtype=f32)
    W2 = np.asarray(inputs["W2"], dtype=f32)
    W3 = np.asarray(inputs["W3"], dtype=f32)

    stu_tbl = np.concatenate(
        [np.asarray(inputs["student_v"], dtype=f32),
         np.asarray(inputs["student_q"], dtype=f32)], axis=1).astype(bf)
    exer_tbl = np.concatenate(
        [np.asarray(inputs["exercise_v"], dtype=f32),
         np.asarray(inputs["exercise_k"], dtype=f32)], axis=1).astype(bf)

    shared = {
        "stu": stu_tbl,
        "exer": exer_tbl,
        "w1aT": np.ascontiguousarray(W1[:, :E].T).astype(bf),
        "w1bT": np.ascontiguousarray(W1[:, E:].T).astype(bf),
        "w2aT": np.ascontiguousarray(W2[:, :E].T).astype(bf),
        "w2bT": np.ascontiguousarray(W2[:, E:].T).astype(bf),
        "w3T": np.ascontiguousarray(W3.T).astype(bf),
        "b1": np.asarray(inputs["b1"], dtype=f32).reshape(E, 1).copy(),
        "b2": np.asarray(inputs["b2"], dtype=f32).reshape(E, 1).copy(),
        "b3t": np.full((KL, 1), np.asarray(inputs["b3"], dtype=f32).reshape(-1)[0], f32),
        "idxS": _wrap_idx(stu_id),
        "idxE": _wrap_idx(exer_id),
    }
    kn = np.asarray(inputs["knowledge_v"], dtype=f32)

    in_maps = []
    for c in range(NCORES):
        m = dict(shared)
        m["knT"] = np.ascontiguousarray(kn[c * KL:(c + 1) * KL, :].T).astype(bf)
        m["kqT"] = np.ascontiguousarray(kq[:, c * KL:(c + 1) * KL].T).astype(bf)
        in_maps.append(m)

    trace = bool(int(os.environ.get("KERNEL_TRACE", "0")))
    ncores = int(os.environ.get("KERNEL_CORES", str(NCORES)))
    res = run_bass_kernel_spmd(nc, in_maps[:ncores], core_ids=list(range(ncores)),
                               trace=trace)
    LAST_RESULTS = res
    acc = np.zeros((2, B), np.float64)
    for c in range(len(res.results)):
        acc += res.results[c]["out"].astype(np.float64)
    return (acc[0] / acc[1]).astype(np.float32)
